# revision 2
# baseline (speedup 1.0000x reference)
"""Compact-prefix attention (nn_Attention_16234976379516) on 8 TRN2 NeuronCores.

Math per (b, h) pair:
    S = (Q @ K^T) * scale          [T, L]
    S[:, :Lc] += beta              (bias on compacted prefix)
    S = where(mask, S, -inf)       (mask folded into bias host-side)
    O = softmax(S, -1) @ V         [T, D]

Device formulation (transposed scores, no on-chip transposes):
    E^T[l, t] = exp(scale * (K Q^T)[l, t])              # PE matmul + exp
    [O*denom | denom] = sum_lc E^T_lc.T @ V'_lc         # PE PSUM accumulation
        where V'[l, :] = e^{bias[l]} * [V[l, :] | 1]    # bias folded into V on
                                                        # host; ones column gives
                                                        # the softmax denominator
    O = (O*denom) * (1/denom)                           # DVE recip + ACT scale

v2: one-head-skewed software pipeline. PE is the bottleneck engine
(~7.0us/head: 32 score MMs of FD=256 + 64 accum MMs of FD=129); the exp
chain (ACT 5.5 tiles + DVE 2.5 tiles, ~6.2-6.3us/head) previously gated
stage-2 within the same head, costing ~2us/head of PE stalls.  Now head
g's period interleaves stage1(g) with stage2(g-1) on PE, so every e chunk
consumed by stage2 was exp'd a full period earlier.

PSUM (8 banks): score tiles 2 banks x 3 bufs + DOUBLE-BUFFERED O
accumulator at 1 bank each ([128, 2, 256] f32; the two t-tile accumulation
groups share the bank SEQUENTIALLY - all 32 tc0 MMs, then all 32 tc1 MMs.
start=True clears the whole bank's has_written bits, which is safe exactly
because the groups don't interleave: tc0's finished VALUES survive, only
its bits are cleared, and nothing accumulates there afterwards).

The exp split across ScalarE (exact Exp, 5.5/8 tiles) and VectorE (bf16
Schraudolph via one tensor_scalar: int16(round(s*A + B)) bitcast as bf16,
2.5/8 tiles on the non-beta l-chunks) is kept from v1, as is the epilogue
(recip + 2 muls) on DVE.  A dummy exp at t=0 pre-loads the ACT exp table
(~2.7us) under the head-0 DMA.

Sharding: (B,H) flattened to 128 pairs, 16 per core (pure H/batch split, no
cross-device communication). Host pre-transposes Q/K per head, pre-tiles V
with the ones column and e^bias (beta + mask) folded in, and casts operands
to bf16 (fp32 PSUM accumulation throughout).
"""

import numpy as np

B, H, T, L, LC, D = 4, 32, 256, 4096, 2048, 128
NCORES = 8
G = B * H                  # 128 (b,h) pairs
GPC = G // NCORES          # 16 pairs per core
N_LC = L // 128            # 32 l-chunks of 128
N_TC = T // 128            # 2 t-chunks of 128
SCALE = 1.0 / float(np.sqrt(D))

# stage1 (scores) operand dtype: fp8e3 (e3m4) halves the kT DMA bytes (the
# steady-state DMA need drops from ~313 GB/s to ~240 GB/s vs the ~358 GB/s
# per-core peak); PE runs fp8 at bf16 speed. Verified numerically on host.
S1_FP8 = False

KT_SPLIT = 2
V1_SPLIT = 2

_NC_CACHE = {}


def build_nc(n_heads=GPC, n_lc=N_LC, n_tc=N_TC):
    """Build the single-core Bass program (run SPMD on all 8 cores)."""
    from contextlib import ExitStack

    import concourse.bacc as bacc
    import concourse.mybir as mybir
    import concourse.tile as tile
    from concourse.bass import ts

    bf16 = mybir.dt.bfloat16
    f8e3 = mybir.dt.float8e3
    f32 = mybir.dt.float32
    i16 = mybir.dt.int16
    s1_dt = f8e3 if S1_FP8 else bf16
    L_ = n_lc * 128
    T_ = n_tc * 128
    ACT_W = 4                  # l-chunks of scores per wide ACT/DVE exp op
    assert n_lc % ACT_W == 0
    n_a = n_lc // ACT_W        # 8 score tiles per head
    # Score-tiles handed to DVE instead of ScalarE (one-op bf16 Schraudolph
    # exp; softmax ratio cancels its mean error; placed on non-beta l-chunks,
    # a >= 4 -> l >= 2048). 2.5 tiles on DVE (tile 6 split by bank: chunks
    # 24,25 -> DVE, 26,27 -> ACT); epilogue also on DVE.
    DVE_TILES = frozenset({4, 5})
    SPLIT_TILE = 6
    A_TS = float(128.0 / np.log(2.0) * SCALE)
    B_TS = 16256.0 - 5.5
    # Stage-1 production order: DVE-destined tiles early (its chain is the
    # slowest per tile), so score bufs recycle evenly.
    A_ORDER = [0, 4, 1, 5, 2, 6, 3, 7]
    # Stage-2 consumption order (per t-tile pass): ACT-produced l-chunks
    # first, DVE-produced last. With the one-head skew everything is ready
    # anyway; this is insurance if an exp chain slips.
    LC_ORDER = (
        [lc for a in (0, 1, 2, 3) for lc in range(a * ACT_W, (a + 1) * ACT_W)]
        + [26, 27]
        + list(range(7 * ACT_W, 8 * ACT_W))
        + list(range(16, 26))
    )
    assert sorted(LC_ORDER) == list(range(n_lc))

    nc = bacc.Bacc("TRN2", target_bir_lowering=False, debug=False)
    qT_d = nc.dram_tensor("qT", [n_heads, 128, T_], s1_dt, kind="ExternalInput").ap()
    kT_d = nc.dram_tensor("kT", [n_heads, 128, L_], s1_dt, kind="ExternalInput").ap()
    # v1[g, p, lc, d]: e^bias[l] * (V row l | 1), l = lc*128 + p
    v1_d = nc.dram_tensor(
        "v1", [n_heads, 128, n_lc, 129], bf16, kind="ExternalInput"
    ).ap()
    out_d = nc.dram_tensor("out", [n_heads, n_tc, 128, 128], f32, kind="ExternalOutput").ap()

    with tile.TileContext(nc) as tc:
        with ExitStack() as ctx:
            in_pool = ctx.enter_context(tc.tile_pool(name="in_pool", bufs=3))
            e_pool = ctx.enter_context(tc.tile_pool(name="e_pool", bufs=2))
            ep_pool = ctx.enter_context(tc.tile_pool(name="ep_pool", bufs=4))
            s_pool = ctx.enter_context(tc.tile_pool(name="s_pool", bufs=3, space="PSUM"))
            o_pool = ctx.enter_context(tc.tile_pool(name="o_pool", bufs=2, space="PSUM"))

            def prime_act():
                # First ACTIVATE triggers the ~2.7us exp table load; issue a
                # dummy at t=0 so it hides under the head-0 kT DMA.
                scrap = ep_pool.tile([128, 1], f32, tag="scrap", name="scrap_sb")
                nc.vector.memset(scrap, 0.0)
                junk = ep_pool.tile([128, 1], bf16, tag="junk", name="junk_sb")
                nc.scalar.activation(
                    out=junk, in_=scrap,
                    func=mybir.ActivationFunctionType.Exp, scale=SCALE,
                )

            def warmup_pe(n_mm=24):
                # The PE clock gate (HAM) starts throttled at 1.2 GHz and
                # needs ~3.4us of sustained busy to lift to 2.4 GHz; the DMA
                # ramp otherwise leaves PE idle/cold for the first ~15us.
                # Burn dummy matmuls on zeroed SBUF tiles (no DMA deps) into
                # a scratch PSUM bank to warm it while head 0 streams in.
                w0 = ep_pool.tile([128, 128], bf16, tag="dw", name="dw_sb")
                nc.vector.memset(w0, 0.0)
                r0 = ep_pool.tile([128, 256], bf16, tag="dr", name="dr_sb")
                nc.vector.memset(r0, 0.0)
                dps = o_pool.tile([128, 256], f32, tag="o", name="o_ps")
                for _ in range(n_mm):
                    nc.tensor.matmul(dps, lhsT=w0, rhs=r0, start=True, stop=True)

            def load_head(g):
                qT = in_pool.tile([128, T_], s1_dt, tag="qT", name="qT_sb")
                nc.sync.dma_start(out=qT, in_=qT_d[g])
                kT = in_pool.tile([128, L_], s1_dt, tag="kT", name="kT_sb")
                if g == 0:
                    # Head 0 gates the pipeline: stream kT in 512-col chunks
                    # in stage-1 consumption order (A_ORDER) over two DMA
                    # queues, so the first matmul waits only for its own
                    # chunk (region-level dependency tracking).
                    w = ACT_W * 128
                    for c, a in enumerate(A_ORDER):
                        q = nc.gpsimd if c % 2 == 0 else nc.sync
                        q.dma_start(
                            out=kT[:, a * w : (a + 1) * w],
                            in_=kT_d[g, :, a * w : (a + 1) * w],
                        )
                else:
                    for c in range(KT_SPLIT):
                        w = L_ // KT_SPLIT
                        nc.sync.dma_start(
                            out=kT[:, c * w : (c + 1) * w],
                            in_=kT_d[g, :, c * w : (c + 1) * w],
                        )
                v1 = in_pool.tile([128, n_lc, 129], bf16, tag="v1", name="v1_sb")
                # v1 on the gpsimd queue: big transfers on a queue of their
                # own, prefetched a full period before first use.
                v1_q = nc.sync if g == 0 else nc.gpsimd
                for c in range(V1_SPLIT):
                    w = n_lc // V1_SPLIT
                    v1_q.dma_start(
                        out=v1[:, c * w : (c + 1) * w, :],
                        in_=v1_d[g, :, c * w : (c + 1) * w, :],
                    )
                return qT, kT, v1

            def dve_exp(e_ap, s_ap):
                nc.vector.tensor_scalar(
                    e_ap.bitcast(i16),
                    s_ap,
                    A_TS,
                    B_TS,
                    op0=mybir.AluOpType.mult,
                    op1=mybir.AluOpType.add,
                )

            def act_exp(e_ap, s_ap):
                nc.scalar.activation(
                    out=e_ap,
                    in_=s_ap,
                    func=mybir.ActivationFunctionType.Exp,
                    scale=SCALE,
                )

            def stage1_tile(qT, kT, e, a):
                """Score matmuls for l-chunks [4a, 4a+4) + exp to e (bf16)."""
                s = s_pool.tile([128, ACT_W, T_], f32, tag="s", name="s_ps")
                for j in range(ACT_W):
                    lc = a * ACT_W + j
                    nc.tensor.matmul(
                        s[:, j, :],
                        lhsT=kT[:, ts(lc, 128)],
                        rhs=qT,
                        start=True,
                        stop=True,
                    )
                e_sl = e[:, a * ACT_W : (a + 1) * ACT_W, :]
                if a in DVE_TILES:
                    dve_exp(e_sl, s)
                elif a == SPLIT_TILE:
                    dve_exp(e[:, a * ACT_W : a * ACT_W + 2, :], s[:, 0:2, :])
                    act_exp(e[:, a * ACT_W + 2 : (a + 1) * ACT_W, :], s[:, 2:4, :])
                else:
                    act_exp(e_sl, s)

            def stage2_part(os2, e, v1, j):
                """Phase j of 8: MMs 8j..8j+8 of the 64-MM accumulation.

                Two sequential 32-MM passes over LC_ORDER (t-tile 0 during
                phases 0-3, t-tile 1 during 4-7), each accumulating into its
                own single-bank tile so the tc0 epilogue can run on DVE
                while PE still accumulates tc1 (different banks)."""
                tci = j // 4
                for i in range((j % 4) * 8, (j % 4) * 8 + 8):
                    lc = LC_ORDER[i]
                    nc.tensor.matmul(
                        os2[tci][:, 0:129],
                        lhsT=e[:, lc, ts(tci, 128)],
                        rhs=v1[:, lc, :],
                        start=(i == 0),
                        stop=(i == n_lc - 1),
                    )

            def epilogue_tc(os_t, g, tci):
                """O = (O*denom)/denom for one t-tile; recip + mul on DVE.

                Issued right after that t-tile's accumulation group closes
                (mid-period), so the final head's epilogue isn't serialized
                behind the very last matmul."""
                recip = ep_pool.tile([128, 1], f32, tag="recip", name="recip_sb")
                nc.vector.reciprocal(recip, os_t[:, 128:129])
                ob = ep_pool.tile([128, 128], f32, tag="ob", name="ob_sb")
                nc.vector.tensor_scalar_mul(ob, os_t[:, 0:128], recip)
                nc.sync.dma_start(out=out_d[g, tci], in_=ob)

            prime_act()
            warmup_pe()
            tiles = {0: load_head(0)}
            es = {}
            for g in range(n_heads):
                if g + 1 < n_heads:
                    tiles[g + 1] = load_head(g + 1)
                qT, kT, v1 = tiles[g]
                e = e_pool.tile([128, n_lc, T_], bf16, tag="e", name="e_sb")
                es[g] = e
                os2 = None
                if g > 0:
                    os2 = [
                        o_pool.tile([128, 256], f32, tag="o", name="o_ps")
                        for _ in range(n_tc)
                    ]
                for j in range(n_a):
                    stage1_tile(qT, kT, e, A_ORDER[j])
                    if g > 0:
                        stage2_part(os2, es[g - 1], tiles[g - 1][2], j)
                        if j == n_a // 2 - 1:
                            epilogue_tc(os2[0], g - 1, 0)
                if g > 0:
                    epilogue_tc(os2[1], g - 1, 1)
                    del tiles[g - 1], es[g - 1]
            # tail: last head's stage2 (all inputs ready; PE runs it densely)
            gl = n_heads - 1
            os2 = [
                o_pool.tile([128, 256], f32, tag="o", name="o_ps")
                for _ in range(n_tc)
            ]
            for j in range(n_a):
                stage2_part(os2, es[gl], tiles[gl][2], j)
                if j == n_a // 2 - 1:
                    epilogue_tc(os2[0], gl, 0)
            epilogue_tc(os2[1], gl, 1)

    nc.compile()
    return nc


def make_core_inputs(q, k, v, beta, attn_mask):
    """Host prep: fold mask+beta into bias, transpose/tile/cast, shard 8 ways.

    Returns list of 8 in_maps (one per core)."""
    import ml_dtypes

    bf16 = ml_dtypes.bfloat16
    s1_np = ml_dtypes.float8_e3m4 if S1_FP8 else bf16

    qf = np.ascontiguousarray(q, np.float32).reshape(G, T, D)
    kf = np.ascontiguousarray(k, np.float32).reshape(G, L, D)
    vf = np.ascontiguousarray(v, np.float32).reshape(G, L, D)

    bias = np.zeros((G, L), np.float32)
    bias[:, :LC] = np.asarray(beta, np.float32).reshape(G, LC)
    mask = np.asarray(attn_mask).reshape(G, L)
    # exp(s + b) = exp(s) * e^b: fold e^bias into the [V | 1] operand so the
    # device exp needs no per-partition bias (enables wide ACT tiles). A
    # masked-out l gets e^-inf = 0, zeroing its numerator+denominator terms.
    ebias = np.where(mask, np.exp(bias), np.float32(0.0))

    in_maps = []
    for i in range(NCORES):
        sl = slice(i * GPC, (i + 1) * GPC)
        qT = np.ascontiguousarray(qf[sl].transpose(0, 2, 1)).astype(s1_np)
        kT = np.ascontiguousarray(kf[sl].transpose(0, 2, 1)).astype(s1_np)
        v1 = np.empty((GPC, L, D + 1), np.float32)
        v1[..., :D] = vf[sl]
        v1[..., D] = 1.0
        v1 *= ebias[sl, :, None]
        v1 = v1.reshape(GPC, N_LC, 128, D + 1).transpose(0, 2, 1, 3)
        in_maps.append(
            {"qT": qT, "kT": kT, "v1": np.ascontiguousarray(v1.astype(bf16))}
        )
    return in_maps


def run_spmd(in_maps, trace=False):
    from concourse import bass_utils

    if "nc" not in _NC_CACHE:
        _NC_CACHE["nc"] = build_nc()
    nc = _NC_CACHE["nc"]
    return bass_utils.run_bass_kernel_spmd(
        nc, in_maps, core_ids=list(range(NCORES)), trace=trace
    )


def kernel(q, k, v, beta, attn_mask):
    res = run_spmd(make_core_inputs(q, k, v, beta, attn_mask))
    out = np.empty((G, T, D), np.float32)
    for i in range(NCORES):
        out[i * GPC : (i + 1) * GPC] = res.results[i]["out"].reshape(GPC, T, D)
    return out.reshape(B, H, T, D)


# revision 7
# speedup vs baseline: 1.1669x; 1.1669x over previous
"""Compact-prefix attention (nn_Attention_16234976379516) on 8 TRN2 NeuronCores.

Math per (b, h) pair:
    S = (Q @ K^T) * scale          [T, L]
    S[:, :Lc] += beta              (bias on compacted prefix)
    S = where(mask, S, -inf)       (mask folded into bias host-side)
    O = softmax(S, -1) @ V         [T, D]

Device formulation (transposed scores, no on-chip transposes):
    E^T[l, t] = exp(scale * (K Q^T)[l, t])              # PE matmul + exp
    [O*denom | denom] = sum_lc E^T_lc.T @ V'_lc         # PE PSUM accumulation
        where V'[l, :] = e^{bias[l]} * [V[l, :] | 1]    # bias folded into V on
                                                        # host; ones column gives
                                                        # the softmax denominator
    O = (O*denom) * (1/denom)                           # DVE recip + ACT scale

v2: one-head-skewed software pipeline. PE is the bottleneck engine
(HW-measured 7.26us/head steady: 32 score MMs of FD=256 at 109.2ns + 64
accum MMs of FD=129 at 58.9ns, 99.9% PE-busy in steady state); the exp
chain (ACT 5.5 tiles + DVE 2.5 tiles, ~6.2-6.3us/head) previously gated
stage-2 within the same head, costing ~2us/head of PE stalls.  Now head
g's period interleaves stage1(g) phases with stage2(g-1) phases on PE, so
every e chunk consumed by stage2 was exp'd a full period earlier.

PSUM (8 banks): score tiles 2 banks x 3 bufs + per-t-tile single-bank O
accumulators ([128, 256] f32, 129 words used), 2 pool slots = effectively
double-buffered across heads.  Each t-tile is its own accumulation group
in its own bank (all 32 tc0 MMs during phases 0-3, tc1 during 4-7), so
the tc0 epilogue (DVE recip + mul, then out-DMA) legally overlaps PE's
tc1 accumulation and the final head's epilogue doesn't trail the last MM.

The exp split across ScalarE (exact Exp, 5.5/8 tiles) and VectorE (bf16
Schraudolph via one tensor_scalar: int16(round(s*A + B)) bitcast as bf16,
2.5/8 tiles on the non-beta l-chunks) is kept from v1.  Ramp: a dummy exp
at t=0 pre-loads the ACT exp table (~2.7us) under the head-0 DMA, and ~24
dummy matmuls on zeroed SBUF tiles warm the PE HAM clock gate (1.2->2.4
GHz needs ~3.4us of sustained busy) while head 0 streams in; without them
the DMA-bound ramp keeps PE cold until ~30us in.

Sharding: (B,H) flattened to 128 pairs, 16 per core (pure H/batch split, no
cross-device communication). Host pre-transposes Q/K per head, pre-tiles V
with the ones column and e^bias (beta + mask) folded in, and casts operands
to bf16 (fp32 PSUM accumulation throughout).
"""

import numpy as np

B, H, T, L, LC, D = 4, 32, 256, 4096, 2048, 128
NCORES = 8
G = B * H                  # 128 (b,h) pairs
GPC = G // NCORES          # 16 pairs per core
N_LC = L // 128            # 32 l-chunks of 128
N_TC = T // 128            # 2 t-chunks of 128
SCALE = 1.0 / float(np.sqrt(D))

# stage1 (scores) operand dtype: fp8e3 (e3m4) would halve the kT DMA bytes,
# but measured accuracy is rel 2.8e-2 vs the 2e-2 gate (CoreSim, q+k both
# e3m4) — too close to the edge; PE gains nothing anyway (fp8 without
# DoubleRow runs at bf16 speed, and DoubleRow needs a 256-deep contraction
# this problem doesn't have). Keep bf16.
S1_FP8 = False

KT_SPLIT = 2
V1_SPLIT = 2

_NC_CACHE = {}


def build_nc(n_heads=GPC, n_lc=N_LC, n_tc=N_TC, warm_mm=24):
    """Build the single-core Bass program (run SPMD on all 8 cores)."""
    from contextlib import ExitStack

    import concourse.bacc as bacc
    import concourse.mybir as mybir
    import concourse.tile as tile
    from concourse.bass import ts

    bf16 = mybir.dt.bfloat16
    f8e3 = mybir.dt.float8e3
    f32 = mybir.dt.float32
    i16 = mybir.dt.int16
    s1_dt = f8e3 if S1_FP8 else bf16
    L_ = n_lc * 128
    T_ = n_tc * 128
    ACT_W = 4                  # l-chunks of scores per wide ACT/DVE exp op
    assert n_lc % ACT_W == 0
    n_a = n_lc // ACT_W        # 8 score tiles per head
    # Score-tiles handed to DVE instead of ScalarE (one-op bf16 Schraudolph
    # exp; softmax ratio cancels its mean error; placed on non-beta l-chunks,
    # a >= 4 -> l >= 2048). 2.5 tiles on DVE (tile 6 split by bank: chunks
    # 24,25 -> DVE, 26,27 -> ACT); epilogue also on DVE.
    DVE_TILES = frozenset({4, 5})
    SPLIT_TILE = 6
    A_TS = float(128.0 / np.log(2.0) * SCALE)
    B_TS = 16256.0 - 5.5
    # Stage-1 production order: DVE-destined tiles early (its chain is the
    # slowest per tile), so score bufs recycle evenly.
    A_ORDER = [0, 4, 1, 5, 2, 6, 3, 7]
    # Stage-2 consumption order (per t-tile pass): ACT-produced l-chunks
    # first, DVE-produced last. With the one-head skew everything is ready
    # anyway; this is insurance if an exp chain slips.
    LC_ORDER = (
        [lc for a in (0, 1, 2, 3) for lc in range(a * ACT_W, (a + 1) * ACT_W)]
        + [26, 27]
        + list(range(7 * ACT_W, 8 * ACT_W))
        + list(range(16, 26))
    )
    assert sorted(LC_ORDER) == list(range(n_lc))

    nc = bacc.Bacc("TRN2", target_bir_lowering=False, debug=False)
    qT_d = nc.dram_tensor("qT", [n_heads, 128, T_], s1_dt, kind="ExternalInput").ap()
    kT_d = nc.dram_tensor("kT", [n_heads, 128, L_], s1_dt, kind="ExternalInput").ap()
    # v1[g, p, lc, d]: e^bias[l] * (V row l | 1), l = lc*128 + p
    v1_d = nc.dram_tensor(
        "v1", [n_heads, 128, n_lc, 129], bf16, kind="ExternalInput"
    ).ap()
    out_d = nc.dram_tensor("out", [n_heads, n_tc, 128, 128], f32, kind="ExternalOutput").ap()

    with tile.TileContext(nc) as tc:
        with ExitStack() as ctx:
            in_pool = ctx.enter_context(tc.tile_pool(name="in_pool", bufs=3))
            e_pool = ctx.enter_context(tc.tile_pool(name="e_pool", bufs=2))
            ep_pool = ctx.enter_context(tc.tile_pool(name="ep_pool", bufs=4))
            s_pool = ctx.enter_context(tc.tile_pool(name="s_pool", bufs=3, space="PSUM"))
            o_pool = ctx.enter_context(tc.tile_pool(name="o_pool", bufs=2, space="PSUM"))

            def prime_act():
                # First ACTIVATE triggers the ~2.7us exp table load; issue a
                # dummy at t=0 so it hides under the head-0 kT DMA.
                scrap = ep_pool.tile([128, 1], f32, tag="scrap", name="scrap_sb")
                nc.vector.memset(scrap, 0.0)
                junk = ep_pool.tile([128, 1], bf16, tag="junk", name="junk_sb")
                nc.scalar.activation(
                    out=junk, in_=scrap,
                    func=mybir.ActivationFunctionType.Exp, scale=SCALE,
                )

            def warmup_pe(n_mm=warm_mm):
                # The PE clock gate (HAM) starts throttled at 1.2 GHz and
                # needs ~3.4us of sustained busy to lift to 2.4 GHz; the DMA
                # ramp otherwise leaves PE idle/cold for the first ~15us.
                # Burn dummy matmuls on zeroed SBUF tiles (no DMA deps) into
                # a scratch PSUM bank to warm it while head 0 streams in.
                w0 = ep_pool.tile([128, 128], bf16, tag="dw", name="dw_sb")
                nc.vector.memset(w0, 0.0)
                r0 = ep_pool.tile([128, 256], bf16, tag="dr", name="dr_sb")
                nc.vector.memset(r0, 0.0)
                dps = o_pool.tile([128, 256], f32, tag="o", name="o_ps")
                for _ in range(n_mm):
                    nc.tensor.matmul(dps, lhsT=w0, rhs=r0, start=True, stop=True)

            def load_head(g):
                qT = in_pool.tile([128, T_], s1_dt, tag="qT", name="qT_sb")
                nc.sync.dma_start(out=qT, in_=qT_d[g])
                kT = in_pool.tile([128, L_], s1_dt, tag="kT", name="kT_sb")
                if g == 0:
                    # Head 0 gates the pipeline: stream kT in 512-col chunks
                    # in stage-1 consumption order (A_ORDER) over two DMA
                    # queues, so the first matmul waits only for its own
                    # chunk (region-level dependency tracking).
                    w = ACT_W * 128
                    for c, a in enumerate(A_ORDER):
                        q = nc.gpsimd if c % 2 == 0 else nc.sync
                        q.dma_start(
                            out=kT[:, a * w : (a + 1) * w],
                            in_=kT_d[g, :, a * w : (a + 1) * w],
                        )
                else:
                    for c in range(KT_SPLIT):
                        w = L_ // KT_SPLIT
                        nc.sync.dma_start(
                            out=kT[:, c * w : (c + 1) * w],
                            in_=kT_d[g, :, c * w : (c + 1) * w],
                        )
                v1 = in_pool.tile([128, n_lc, 129], bf16, tag="v1", name="v1_sb")
                # v1 on the gpsimd queue: big transfers on a queue of their
                # own, prefetched a full period before first use.
                v1_q = nc.sync if g == 0 else nc.gpsimd
                for c in range(V1_SPLIT):
                    w = n_lc // V1_SPLIT
                    v1_q.dma_start(
                        out=v1[:, c * w : (c + 1) * w, :],
                        in_=v1_d[g, :, c * w : (c + 1) * w, :],
                    )
                return qT, kT, v1

            def dve_exp(e_ap, s_ap):
                nc.vector.tensor_scalar(
                    e_ap.bitcast(i16),
                    s_ap,
                    A_TS,
                    B_TS,
                    op0=mybir.AluOpType.mult,
                    op1=mybir.AluOpType.add,
                )

            def act_exp(e_ap, s_ap):
                nc.scalar.activation(
                    out=e_ap,
                    in_=s_ap,
                    func=mybir.ActivationFunctionType.Exp,
                    scale=SCALE,
                )

            def stage1_tile(qT, kT, e, a):
                """Score matmuls for l-chunks [4a, 4a+4) + exp to e (bf16)."""
                s = s_pool.tile([128, ACT_W, T_], f32, tag="s", name="s_ps")
                for j in range(ACT_W):
                    lc = a * ACT_W + j
                    nc.tensor.matmul(
                        s[:, j, :],
                        lhsT=kT[:, ts(lc, 128)],
                        rhs=qT,
                        start=True,
                        stop=True,
                    )
                e_sl = e[:, a * ACT_W : (a + 1) * ACT_W, :]
                if a in DVE_TILES:
                    dve_exp(e_sl, s)
                elif a == SPLIT_TILE:
                    dve_exp(e[:, a * ACT_W : a * ACT_W + 2, :], s[:, 0:2, :])
                    act_exp(e[:, a * ACT_W + 2 : (a + 1) * ACT_W, :], s[:, 2:4, :])
                else:
                    act_exp(e_sl, s)

            def stage2_part(os2, e, v1, j):
                """Phase j of 8: MMs 8j..8j+8 of the 64-MM accumulation.

                Two sequential 32-MM passes over LC_ORDER (t-tile 0 during
                phases 0-3, t-tile 1 during 4-7), each accumulating into its
                own single-bank tile so the tc0 epilogue can run on DVE
                while PE still accumulates tc1 (different banks)."""
                tci = j // 4
                for i in range((j % 4) * 8, (j % 4) * 8 + 8):
                    lc = LC_ORDER[i]
                    nc.tensor.matmul(
                        os2[tci][:, 0:129],
                        lhsT=e[:, lc, ts(tci, 128)],
                        rhs=v1[:, lc, :],
                        start=(i == 0),
                        stop=(i == n_lc - 1),
                    )

            def epilogue_tc(os_t, g, tci):
                """O = (O*denom)/denom for one t-tile; recip + mul on DVE.

                Issued right after that t-tile's accumulation group closes
                (mid-period), so the final head's epilogue isn't serialized
                behind the very last matmul."""
                recip = ep_pool.tile([128, 1], f32, tag="recip", name="recip_sb")
                nc.vector.reciprocal(recip, os_t[:, 128:129])
                ob = ep_pool.tile([128, 128], f32, tag="ob", name="ob_sb")
                nc.vector.tensor_scalar_mul(ob, os_t[:, 0:128], recip)
                nc.sync.dma_start(out=out_d[g, tci], in_=ob)

            prime_act()
            warmup_pe()
            tiles = {0: load_head(0)}
            es = {}
            for g in range(n_heads):
                if g + 1 < n_heads:
                    tiles[g + 1] = load_head(g + 1)
                qT, kT, v1 = tiles[g]
                e = e_pool.tile([128, n_lc, T_], bf16, tag="e", name="e_sb")
                es[g] = e
                os2 = None
                if g > 0:
                    os2 = [
                        o_pool.tile([128, 256], f32, tag="o", name="o_ps")
                        for _ in range(n_tc)
                    ]
                for j in range(n_a):
                    stage1_tile(qT, kT, e, A_ORDER[j])
                    if g > 0:
                        stage2_part(os2, es[g - 1], tiles[g - 1][2], j)
                        if j == n_a // 2 - 1:
                            epilogue_tc(os2[0], g - 1, 0)
                if g > 0:
                    epilogue_tc(os2[1], g - 1, 1)
                    del tiles[g - 1], es[g - 1]
            # tail: last head's stage2 (all inputs ready; PE runs it densely)
            gl = n_heads - 1
            os2 = [
                o_pool.tile([128, 256], f32, tag="o", name="o_ps")
                for _ in range(n_tc)
            ]
            for j in range(n_a):
                stage2_part(os2, es[gl], tiles[gl][2], j)
                if j == n_a // 2 - 1:
                    epilogue_tc(os2[0], gl, 0)
            epilogue_tc(os2[1], gl, 1)

    nc.compile()
    return nc


def make_core_inputs(q, k, v, beta, attn_mask):
    """Host prep: fold mask+beta into bias, transpose/tile/cast, shard 8 ways.

    Returns list of 8 in_maps (one per core)."""
    import ml_dtypes

    bf16 = ml_dtypes.bfloat16
    s1_np = ml_dtypes.float8_e3m4 if S1_FP8 else bf16

    qf = np.ascontiguousarray(q, np.float32).reshape(G, T, D)
    kf = np.ascontiguousarray(k, np.float32).reshape(G, L, D)
    vf = np.ascontiguousarray(v, np.float32).reshape(G, L, D)

    bias = np.zeros((G, L), np.float32)
    bias[:, :LC] = np.asarray(beta, np.float32).reshape(G, LC)
    mask = np.asarray(attn_mask).reshape(G, L)
    # exp(s + b) = exp(s) * e^b: fold e^bias into the [V | 1] operand so the
    # device exp needs no per-partition bias (enables wide ACT tiles). A
    # masked-out l gets e^-inf = 0, zeroing its numerator+denominator terms.
    ebias = np.where(mask, np.exp(bias), np.float32(0.0))

    in_maps = []
    for i in range(NCORES):
        sl = slice(i * GPC, (i + 1) * GPC)
        qT = np.ascontiguousarray(qf[sl].transpose(0, 2, 1)).astype(s1_np)
        kT = np.ascontiguousarray(kf[sl].transpose(0, 2, 1)).astype(s1_np)
        v1 = np.empty((GPC, L, D + 1), np.float32)
        v1[..., :D] = vf[sl]
        v1[..., D] = 1.0
        v1 *= ebias[sl, :, None]
        v1 = v1.reshape(GPC, N_LC, 128, D + 1).transpose(0, 2, 1, 3)
        in_maps.append(
            {"qT": qT, "kT": kT, "v1": np.ascontiguousarray(v1.astype(bf16))}
        )
    return in_maps


def run_spmd(in_maps, trace=False):
    from concourse import bass_utils

    if "nc" not in _NC_CACHE:
        _NC_CACHE["nc"] = build_nc()
    nc = _NC_CACHE["nc"]
    return bass_utils.run_bass_kernel_spmd(
        nc, in_maps, core_ids=list(range(NCORES)), trace=trace
    )


def kernel(q, k, v, beta, attn_mask):
    res = run_spmd(make_core_inputs(q, k, v, beta, attn_mask))
    out = np.empty((G, T, D), np.float32)
    for i in range(NCORES):
        out[i * GPC : (i + 1) * GPC] = res.results[i]["out"].reshape(GPC, T, D)
    return out.reshape(B, H, T, D)


# revision 8
# speedup vs baseline: 1.1924x; 1.0218x over previous
"""Compact-prefix attention (nn_Attention_16234976379516) on 8 TRN2 NeuronCores.

Math per (b, h) pair:
    S = (Q @ K^T) * scale          [T, L]
    S[:, :Lc] += beta              (bias on compacted prefix)
    S = where(mask, S, -inf)       (mask folded into bias host-side)
    O = softmax(S, -1) @ V         [T, D]

Device formulation (transposed scores, no on-chip transposes):
    E^T[l, t] = exp(scale * (K Q^T)[l, t])              # PE matmul + exp
    [O*denom | denom] = sum_lc E^T_lc.T @ V'_lc         # PE PSUM accumulation
        where V'[l, :] = e^{bias[l]} * [V[l, :] | 1]    # bias folded into V on
                                                        # host; ones column gives
                                                        # the softmax denominator
    O = (O*denom) * (1/denom)                           # DVE recip + ACT scale

v2: one-head-skewed software pipeline. PE is the bottleneck engine
(HW-measured 7.26us/head steady: 32 score MMs of FD=256 at 109.2ns + 64
accum MMs of FD=129 at 58.9ns, 99.9% PE-busy in steady state); the exp
chain (ACT 5.5 tiles + DVE 2.5 tiles, ~6.2-6.3us/head) previously gated
stage-2 within the same head, costing ~2us/head of PE stalls.  Now head
g's period interleaves stage1(g) phases with stage2(g-1) phases on PE, so
every e chunk consumed by stage2 was exp'd a full period earlier.

PSUM (8 banks): score tiles 2 banks x 3 bufs + per-t-tile single-bank O
accumulators ([128, 256] f32, 129 words used), 2 pool slots = effectively
double-buffered across heads.  Each t-tile is its own accumulation group
in its own bank (all 32 tc0 MMs during phases 0-3, tc1 during 4-7), so
the tc0 epilogue (DVE recip + mul, then out-DMA) legally overlaps PE's
tc1 accumulation and the final head's epilogue doesn't trail the last MM.

The exp split across ScalarE (exact Exp, 5.5/8 tiles) and VectorE (bf16
Schraudolph via one tensor_scalar: int16(round(s*A + B)) bitcast as bf16,
2.5/8 tiles on the non-beta l-chunks) is kept from v1.  Ramp: a dummy exp
at t=0 pre-loads the ACT exp table (~2.7us) under the head-0 DMA, and ~24
dummy matmuls on zeroed SBUF tiles warm the PE HAM clock gate (1.2->2.4
GHz needs ~3.4us of sustained busy) while head 0 streams in; without them
the DMA-bound ramp keeps PE cold until ~30us in.

Sharding: (B,H) flattened to 128 pairs, 16 per core (pure H/batch split, no
cross-device communication). Host pre-transposes Q/K per head, pre-tiles V
with the ones column and e^bias (beta + mask) folded in, and casts operands
to bf16 (fp32 PSUM accumulation throughout).
"""

import numpy as np

B, H, T, L, LC, D = 4, 32, 256, 4096, 2048, 128
NCORES = 8
G = B * H                  # 128 (b,h) pairs
GPC = G // NCORES          # 16 pairs per core
N_LC = L // 128            # 32 l-chunks of 128
N_TC = T // 128            # 2 t-chunks of 128
SCALE = 1.0 / float(np.sqrt(D))

# stage1 (scores) operand dtype: fp8e3 (e3m4) would halve the kT DMA bytes,
# but measured accuracy is rel 2.8e-2 vs the 2e-2 gate (CoreSim, q+k both
# e3m4) — too close to the edge; PE gains nothing anyway (fp8 without
# DoubleRow runs at bf16 speed, and DoubleRow needs a 256-deep contraction
# this problem doesn't have). Keep bf16.
S1_FP8 = False

KT_SPLIT = 2
V1_SPLIT = 2

_NC_CACHE = {}


def build_nc(n_heads=GPC, n_lc=N_LC, n_tc=N_TC, warm_mm=24):
    """Build the single-core Bass program (run SPMD on all 8 cores)."""
    from contextlib import ExitStack

    import concourse.bacc as bacc
    import concourse.mybir as mybir
    import concourse.tile as tile
    from concourse.bass import ts

    bf16 = mybir.dt.bfloat16
    f8e3 = mybir.dt.float8e3
    f32 = mybir.dt.float32
    i16 = mybir.dt.int16
    s1_dt = f8e3 if S1_FP8 else bf16
    L_ = n_lc * 128
    T_ = n_tc * 128
    ACT_W = 4                  # l-chunks of scores per wide ACT/DVE exp op
    assert n_lc % ACT_W == 0
    n_a = n_lc // ACT_W        # 8 score tiles per head
    # Score-tiles handed to DVE instead of ScalarE (one-op bf16 Schraudolph
    # exp; softmax ratio cancels its mean error; placed on non-beta l-chunks,
    # a >= 4 -> l >= 2048). 2.5 tiles on DVE (tile 6 split by bank: chunks
    # 24,25 -> DVE, 26,27 -> ACT); epilogue also on DVE.
    DVE_TILES = frozenset({4, 5})
    SPLIT_TILE = 6
    A_TS = float(128.0 / np.log(2.0) * SCALE)
    B_TS = 16256.0 - 5.5
    # Stage-1 production order: DVE-destined tiles early (its chain is the
    # slowest per tile), so score bufs recycle evenly.
    A_ORDER = [0, 4, 1, 5, 2, 6, 3, 7]
    # Stage-2 consumption order (per t-tile pass): ACT-produced l-chunks
    # first, DVE-produced last. With the one-head skew everything is ready
    # anyway; this is insurance if an exp chain slips.
    LC_ORDER = (
        [lc for a in (0, 1, 2, 3) for lc in range(a * ACT_W, (a + 1) * ACT_W)]
        + [26, 27]
        + list(range(7 * ACT_W, 8 * ACT_W))
        + list(range(16, 26))
    )
    assert sorted(LC_ORDER) == list(range(n_lc))

    nc = bacc.Bacc("TRN2", target_bir_lowering=False, debug=False)
    qT_d = nc.dram_tensor("qT", [n_heads, 128, T_], s1_dt, kind="ExternalInput").ap()
    kT_d = nc.dram_tensor("kT", [n_heads, 128, L_], s1_dt, kind="ExternalInput").ap()
    # v1[g, p, lc, d]: e^bias[l] * (V row l | 1), l = lc*128 + p
    v1_d = nc.dram_tensor(
        "v1", [n_heads, 128, n_lc, 129], bf16, kind="ExternalInput"
    ).ap()
    out_d = nc.dram_tensor("out", [n_heads, n_tc, 128, 128], f32, kind="ExternalOutput").ap()

    with tile.TileContext(nc) as tc:
        with ExitStack() as ctx:
            in_pool = ctx.enter_context(tc.tile_pool(name="in_pool", bufs=3))
            e_pool = ctx.enter_context(tc.tile_pool(name="e_pool", bufs=2))
            ep_pool = ctx.enter_context(tc.tile_pool(name="ep_pool", bufs=4))
            s_pool = ctx.enter_context(tc.tile_pool(name="s_pool", bufs=3, space="PSUM"))
            o_pool = ctx.enter_context(tc.tile_pool(name="o_pool", bufs=2, space="PSUM"))

            def prime_act():
                # First ACTIVATE triggers the ~2.7us exp table load; issue a
                # dummy at t=0 so it hides under the head-0 kT DMA.
                scrap = ep_pool.tile([128, 1], f32, tag="scrap", name="scrap_sb")
                nc.vector.memset(scrap, 0.0)
                junk = ep_pool.tile([128, 1], bf16, tag="junk", name="junk_sb")
                nc.scalar.activation(
                    out=junk, in_=scrap,
                    func=mybir.ActivationFunctionType.Exp, scale=SCALE,
                )

            def warmup_pe(n_mm=warm_mm):
                # The PE clock gate (HAM) starts throttled at 1.2 GHz and
                # needs ~3.4us of sustained busy to lift to 2.4 GHz; the DMA
                # ramp otherwise leaves PE idle/cold for the first ~15us.
                # Burn dummy matmuls on zeroed SBUF tiles (no DMA deps) into
                # a scratch PSUM bank to warm it while head 0 streams in.
                w0 = ep_pool.tile([128, 128], bf16, tag="dw", name="dw_sb")
                nc.vector.memset(w0, 0.0)
                r0 = ep_pool.tile([128, 256], bf16, tag="dr", name="dr_sb")
                nc.vector.memset(r0, 0.0)
                dps = o_pool.tile([128, 256], f32, tag="o", name="o_ps")
                for _ in range(n_mm):
                    nc.tensor.matmul(dps, lhsT=w0, rhs=r0, start=True, stop=True)

            def load_head(g):
                qT = in_pool.tile([128, T_], s1_dt, tag="qT", name="qT_sb")
                nc.sync.dma_start(out=qT, in_=qT_d[g])
                kT = in_pool.tile([128, L_], s1_dt, tag="kT", name="kT_sb")
                if g == 0:
                    # Head 0 gates the pipeline: stream kT in 512-col chunks
                    # in stage-1 consumption order (A_ORDER) over two DMA
                    # queues, so the first matmul waits only for its own
                    # chunk (region-level dependency tracking).
                    w = ACT_W * 128
                    for c, a in enumerate(A_ORDER):
                        q = nc.gpsimd if c % 2 == 0 else nc.sync
                        q.dma_start(
                            out=kT[:, a * w : (a + 1) * w],
                            in_=kT_d[g, :, a * w : (a + 1) * w],
                        )
                else:
                    for c in range(KT_SPLIT):
                        w = L_ // KT_SPLIT
                        nc.sync.dma_start(
                            out=kT[:, c * w : (c + 1) * w],
                            in_=kT_d[g, :, c * w : (c + 1) * w],
                        )
                v1 = in_pool.tile([128, n_lc, 129], bf16, tag="v1", name="v1_sb")
                # v1 always on the gpsimd queue (behind head-0's even kT
                # chunks): the sync queue then delivers qT1+kT1 without a
                # 1MB v1 transfer in front, and v1(0) still lands in time
                # for period 1 on the other queue.
                v1_q = nc.gpsimd
                for c in range(V1_SPLIT):
                    w = n_lc // V1_SPLIT
                    v1_q.dma_start(
                        out=v1[:, c * w : (c + 1) * w, :],
                        in_=v1_d[g, :, c * w : (c + 1) * w, :],
                    )
                return qT, kT, v1

            def dve_exp(e_ap, s_ap):
                nc.vector.tensor_scalar(
                    e_ap.bitcast(i16),
                    s_ap,
                    A_TS,
                    B_TS,
                    op0=mybir.AluOpType.mult,
                    op1=mybir.AluOpType.add,
                )

            def act_exp(e_ap, s_ap):
                nc.scalar.activation(
                    out=e_ap,
                    in_=s_ap,
                    func=mybir.ActivationFunctionType.Exp,
                    scale=SCALE,
                )

            def stage1_tile(qT, kT, e, a):
                """Score matmuls for l-chunks [4a, 4a+4) + exp to e (bf16)."""
                s = s_pool.tile([128, ACT_W, T_], f32, tag="s", name="s_ps")
                for j in range(ACT_W):
                    lc = a * ACT_W + j
                    nc.tensor.matmul(
                        s[:, j, :],
                        lhsT=kT[:, ts(lc, 128)],
                        rhs=qT,
                        start=True,
                        stop=True,
                    )
                e_sl = e[:, a * ACT_W : (a + 1) * ACT_W, :]
                if a in DVE_TILES:
                    dve_exp(e_sl, s)
                elif a == SPLIT_TILE:
                    dve_exp(e[:, a * ACT_W : a * ACT_W + 2, :], s[:, 0:2, :])
                    act_exp(e[:, a * ACT_W + 2 : (a + 1) * ACT_W, :], s[:, 2:4, :])
                else:
                    act_exp(e_sl, s)

            def stage2_part(os2, e, v1, j):
                """Phase j of 8: MMs 8j..8j+8 of the 64-MM accumulation.

                Two sequential 32-MM passes over LC_ORDER (t-tile 0 during
                phases 0-3, t-tile 1 during 4-7), each accumulating into its
                own single-bank tile so the tc0 epilogue can run on DVE
                while PE still accumulates tc1 (different banks)."""
                tci = j // 4
                for i in range((j % 4) * 8, (j % 4) * 8 + 8):
                    lc = LC_ORDER[i]
                    nc.tensor.matmul(
                        os2[tci][:, 0:129],
                        lhsT=e[:, lc, ts(tci, 128)],
                        rhs=v1[:, lc, :],
                        start=(i == 0),
                        stop=(i == n_lc - 1),
                    )

            def epilogue_tc(os_t, g, tci):
                """O = (O*denom)/denom for one t-tile; recip + mul on DVE.

                Issued right after that t-tile's accumulation group closes
                (mid-period), so the final head's epilogue isn't serialized
                behind the very last matmul."""
                recip = ep_pool.tile([128, 1], f32, tag="recip", name="recip_sb")
                nc.vector.reciprocal(recip, os_t[:, 128:129])
                ob = ep_pool.tile([128, 128], f32, tag="ob", name="ob_sb")
                nc.vector.tensor_scalar_mul(ob, os_t[:, 0:128], recip)
                nc.sync.dma_start(out=out_d[g, tci], in_=ob)

            prime_act()
            warmup_pe()
            tiles = {0: load_head(0)}
            es = {}
            for g in range(n_heads):
                if g + 1 < n_heads:
                    tiles[g + 1] = load_head(g + 1)
                qT, kT, v1 = tiles[g]
                e = e_pool.tile([128, n_lc, T_], bf16, tag="e", name="e_sb")
                es[g] = e
                os2 = None
                if g > 0:
                    os2 = [
                        o_pool.tile([128, 256], f32, tag="o", name="o_ps")
                        for _ in range(n_tc)
                    ]
                for j in range(n_a):
                    stage1_tile(qT, kT, e, A_ORDER[j])
                    if g > 0:
                        stage2_part(os2, es[g - 1], tiles[g - 1][2], j)
                        if j == n_a // 2 - 1:
                            epilogue_tc(os2[0], g - 1, 0)
                if g > 0:
                    epilogue_tc(os2[1], g - 1, 1)
                    del tiles[g - 1], es[g - 1]
            # tail: last head's stage2 (all inputs ready; PE runs it densely)
            gl = n_heads - 1
            os2 = [
                o_pool.tile([128, 256], f32, tag="o", name="o_ps")
                for _ in range(n_tc)
            ]
            for j in range(n_a):
                stage2_part(os2, es[gl], tiles[gl][2], j)
                if j == n_a // 2 - 1:
                    epilogue_tc(os2[0], gl, 0)
            epilogue_tc(os2[1], gl, 1)

    nc.compile()
    return nc


def make_core_inputs(q, k, v, beta, attn_mask):
    """Host prep: fold mask+beta into bias, transpose/tile/cast, shard 8 ways.

    Returns list of 8 in_maps (one per core)."""
    import ml_dtypes

    bf16 = ml_dtypes.bfloat16
    s1_np = ml_dtypes.float8_e3m4 if S1_FP8 else bf16

    qf = np.ascontiguousarray(q, np.float32).reshape(G, T, D)
    kf = np.ascontiguousarray(k, np.float32).reshape(G, L, D)
    vf = np.ascontiguousarray(v, np.float32).reshape(G, L, D)

    bias = np.zeros((G, L), np.float32)
    bias[:, :LC] = np.asarray(beta, np.float32).reshape(G, LC)
    mask = np.asarray(attn_mask).reshape(G, L)
    # exp(s + b) = exp(s) * e^b: fold e^bias into the [V | 1] operand so the
    # device exp needs no per-partition bias (enables wide ACT tiles). A
    # masked-out l gets e^-inf = 0, zeroing its numerator+denominator terms.
    ebias = np.where(mask, np.exp(bias), np.float32(0.0))

    in_maps = []
    for i in range(NCORES):
        sl = slice(i * GPC, (i + 1) * GPC)
        qT = np.ascontiguousarray(qf[sl].transpose(0, 2, 1)).astype(s1_np)
        kT = np.ascontiguousarray(kf[sl].transpose(0, 2, 1)).astype(s1_np)
        v1 = np.empty((GPC, L, D + 1), np.float32)
        v1[..., :D] = vf[sl]
        v1[..., D] = 1.0
        v1 *= ebias[sl, :, None]
        v1 = v1.reshape(GPC, N_LC, 128, D + 1).transpose(0, 2, 1, 3)
        in_maps.append(
            {"qT": qT, "kT": kT, "v1": np.ascontiguousarray(v1.astype(bf16))}
        )
    return in_maps


def run_spmd(in_maps, trace=False):
    from concourse import bass_utils

    if "nc" not in _NC_CACHE:
        _NC_CACHE["nc"] = build_nc()
    nc = _NC_CACHE["nc"]
    return bass_utils.run_bass_kernel_spmd(
        nc, in_maps, core_ids=list(range(NCORES)), trace=trace
    )


def kernel(q, k, v, beta, attn_mask):
    res = run_spmd(make_core_inputs(q, k, v, beta, attn_mask))
    out = np.empty((G, T, D), np.float32)
    for i in range(NCORES):
        out[i * GPC : (i + 1) * GPC] = res.results[i]["out"].reshape(GPC, T, D)
    return out.reshape(B, H, T, D)
